# revision 2
# baseline (speedup 1.0000x reference)
"""Trainium2 Bass kernel for DatasetIndexedTopK (streaming top-k retrieval).

Problem: scores = Q @ C^T with Q [512, 128], C [2^20, 128]; return per-query
top-100 (scores, ids), matching jax.lax.top_k semantics (ties -> lower id).

Design ("PACK", 8-way candidate shard, one NeuronCore each):

  * One fp8-e4m3 DoubleRow matmul per [128-query x 512-candidate] block with
    K=132: channels 0..127 carry integer-quantized embeddings (query side
    16*round(2q), exact multiples of 16 <= 128; candidate side round(2c),
    ints <= |16|), channels 128..131 carry the candidate's position within
    its 2048-wide PSUM span as 4 base-8 digits against query-side weights
    [2, 2^-2, 2^-5, 2^-8] (all e4m3-exact).  PSUM then holds, exactly in
    fp32 (verified bit-exact on HW),

        y = 16*score' + pos/256,   score' = sum round(2q_d)*round(2c_d)

    i.e. the quantized score AND the in-span position packed in one number.
  * A single DVE top-8 `max` instruction per [128, 2048] PSUM span writes the
    8 largest y per query into the summary S -- no max_index pass, no
    PSUM->SBUF staging, no in-kernel extraction.  Measured on HW: Max reads
    PSUM at ~0.5 ns/elem, 2x the SBUF-source rate, so reading PSUM directly
    is optimal; the scalar engine stays idle on purpose.
  * Per core that is 64 spans x 4 query chunks = 256 units; DVE is the
    bottleneck engine (~1.1-1.4 us/unit incl. PE-write contention), PE fp8
    DoubleRow ~0.6 us/unit, candidate DMA (17 MB fp8) fully hidden.

Host: decodes (score', pos) from the 8 x 512 x 8 = 4096 packed survivors per
query (span index from the S slot, position from the fraction bits), takes
the top-2048 by quantized score (quantization noise sigma ~2.3 vs the >20
score-unit margin makes deeper misses impossible), recomputes exact fp32
scores for those, and emits the exact top-k ordered like the reference
(ties -> lower id).  Quantized scores are bias-free: clip bounds (+-8 query
at step .5 = 4 sigma on ~4 of 65536 coords; +-16 candidate = 8 sigma, never
binds) were chosen after clip bias at +-8 on the candidate side measurably
cost rank misses.  All fp8 plane values stay <= 128: HW e4m3 treats the max
exponent as special (values > 240 come back NaN), unlike ml_dtypes e4m3fn.
"""

import numpy as np

P = 128                  # queries per chunk / PSUM partitions
D = 128                  # embedding dim
Q = 512                  # queries
NCORES = 8
NCAND_TOTAL = 256 * 4096
NCAND = NCAND_TOTAL // NCORES     # 131072 candidates per core
SPAN = 2048                       # candidates per PSUM span (4 banks)
NSPAN = NCAND // SPAN             # 64 spans per core
CTILE = 8192                      # candidate columns per DMA tile
NCHUNK = Q // P                   # 4 query chunks
KH = 66                           # half-K partitions (132 = 128 emb + 4 digit channels)
S_W = NSPAN * 8                   # 512 summary slots per chunk

# query-side digit weights for pos = d3*512 + d2*64 + d1*8 + d0 -> pos/256
POSW = (2.0, 0.25, 2.0 ** -5, 2.0 ** -8)

_CACHE = {}


def _build_bass(cand_bufs=3, psum_bufs=2, repeat=1):
    import concourse.bacc as bacc
    import concourse.mybir as mybir
    from concourse.tile import TileContext
    from contextlib import ExitStack

    f32 = mybir.dt.float32
    fp8 = mybir.dt.float8e4
    DR = mybir.MatmulPerfMode.DoubleRow

    nc = bacc.Bacc()
    qT = nc.declare_dram_parameter("qT", [KH, 2 * Q], fp8, isOutput=False)
    candT = nc.declare_dram_parameter("candT", [KH, 2 * NCAND], fp8, isOutput=False)
    out_s = nc.declare_dram_parameter("out_s", [Q, S_W], f32, isOutput=True)

    with ExitStack() as ctx:
        tc = ctx.enter_context(TileContext(nc))
        qpool = ctx.enter_context(tc.tile_pool(name="q", bufs=1))
        cpool = ctx.enter_context(tc.tile_pool(name="cand", bufs=cand_bufs))
        pspool = ctx.enter_context(tc.tile_pool(name="ps", bufs=psum_bufs, space="PSUM"))
        acc = ctx.enter_context(tc.tile_pool(name="acc", bufs=1))

        qsb = qpool.tile([KH, 2, Q], fp8, tag="qsb")
        nc.sync.dma_start(qsb[:], qT.rearrange("k (two q) -> k two q", two=2))

        S_all = acc.tile([P, NCHUNK * S_W], f32, tag="S")

        candT3 = candT.rearrange("k (two c) -> k two c", two=2)
        for t in range(repeat * (NCAND // CTILE)):
            t = t % (NCAND // CTILE)
            ct = cpool.tile([KH, 2, CTILE], fp8, tag="cand")
            nc.sync.dma_start(ct[:], candT3[:, :, t * CTILE:(t + 1) * CTILE])
            for sp in range(CTILE // SPAN):
                g = t * (CTILE // SPAN) + sp          # global span index
                for qc in range(NCHUNK):
                    ps = pspool.tile([P, SPAN], f32, tag="ps")
                    for j in range(SPAN // 512):
                        col = sp * SPAN + j * 512
                        nc.tensor.matmul(
                            ps[:, j * 512:(j + 1) * 512],
                            lhsT=qsb[:, :, qc * P:(qc + 1) * P],
                            rhs=ct[:, :, col:col + 512],
                            start=True, stop=True,
                            perf_mode=DR,
                        )
                    so = qc * S_W + g * 8
                    nc.vector.max(out=S_all[:, so:so + 8], in_=ps[:])

        for qc in range(NCHUNK):
            nc.sync.dma_start(
                out_s[qc * P:(qc + 1) * P, :],
                S_all[:, qc * S_W:(qc + 1) * S_W],
            )
    nc.compile()
    return nc


def _get_bass():
    if "nc" not in _CACHE:
        _CACHE["nc"] = _build_bass()
    return _CACHE["nc"]


def _host_pack(q, c):
    """Build the fp8 input planes.  q [Q,128] f32, c [NCAND_TOTAL,128] f32."""
    import ml_dtypes
    f8 = ml_dtypes.float8_e4m3fn

    qi = np.round(2.0 * q)
    np.clip(qi, -8, 8, out=qi)
    ci = np.round(2.0 * c)
    np.clip(ci, -16, 16, out=ci)

    qpl = np.zeros((2 * KH, Q), np.float32)
    qpl[:D] = 16.0 * qi.T                       # multiples of 16 <= 128, exact
    for i, w in enumerate(POSW):
        qpl[D + i] = w
    qT = np.ascontiguousarray(qpl.reshape(KH, 2, Q).reshape(KH, 2 * Q))

    pos = np.arange(SPAN, dtype=np.int64)
    digs = np.stack([(pos >> 9) & 7, (pos >> 6) & 7, (pos >> 3) & 7, pos & 7])
    digs = np.tile(digs, (1, NCAND // SPAN)).astype(np.float32)  # [4, NCAND]

    cores = []
    for core in range(NCORES):
        shard = ci[core * NCAND:(core + 1) * NCAND]          # [NCAND, 128]
        cpl = np.empty((2 * KH, NCAND), np.float32)
        cpl[:D] = shard.T
        cpl[D:D + 4] = digs
        # channel 2*k2+j lives at plane [k2, j, :]  (DoubleRow pair-outer)
        cores.append(np.ascontiguousarray(cpl).reshape(KH, 2 * NCAND).astype(f8))
    return qT.astype(f8), cores


def kernel(query_embeddings, candidate_embeddings, candidate_indices, k):
    from concourse.bass_utils import run_bass_kernel_spmd

    q = np.ascontiguousarray(np.asarray(query_embeddings, dtype=np.float32))
    c = np.asarray(candidate_embeddings, dtype=np.float32).reshape(NCAND_TOTAL, D)
    ids_flat = np.asarray(candidate_indices).reshape(-1)
    k = int(k)

    qT8, cand8 = _host_pack(q, c)
    in_maps = [{"qT": qT8, "candT": cand8[core]} for core in range(NCORES)]

    nc = _get_bass()
    res = run_bass_kernel_spmd(nc, in_maps, core_ids=list(range(NCORES))).results

    # ---- host decode: y = 16*score' + pos/256, slot s -> span s>>3 ----
    n_per_core = S_W
    all_s4 = np.empty((Q, NCORES * n_per_core), np.float32)   # score' (int)
    all_gp = np.empty((Q, NCORES * n_per_core), np.int64)     # global position
    span_of_slot = (np.arange(S_W) >> 3).astype(np.int64)
    for core in range(NCORES):
        y = res[core]["out_s"].astype(np.float64)             # [Q, 512]
        s16 = np.floor(y / 16.0)
        pos = np.rint((y - 16.0 * s16) * 256.0).astype(np.int64)
        np.clip(pos, 0, SPAN - 1, out=pos)
        gp = core * NCAND + span_of_slot[None, :] * SPAN + pos
        sl = slice(core * n_per_core, (core + 1) * n_per_core)
        all_s4[:, sl] = s16
        all_gp[:, sl] = gp

    # top-T by quantized score, then exact fp32 rescore (chunked)
    T = min(2048, all_s4.shape[1])
    idx = np.argpartition(-all_s4, T - 1, axis=1)[:, :T]
    gp_top = np.take_along_axis(all_gp, idx, axis=1)          # [Q, T]

    exact = np.empty((Q, T), np.float32)
    QB = 128
    for q0 in range(0, Q, QB):
        emb = c[gp_top[q0:q0 + QB]]                           # [QB, T, 128]
        exact[q0:q0 + QB] = np.einsum(
            "qd,qtd->qt", q[q0:q0 + QB], emb, optimize=True)

    out_scores = np.empty((Q, k), dtype=np.float32)
    out_pos = np.empty((Q, k), dtype=np.int64)
    for qi_ in range(Q):
        order = np.lexsort((gp_top[qi_], -exact[qi_]))[:k]
        out_scores[qi_] = exact[qi_, order]
        out_pos[qi_] = gp_top[qi_, order]

    out_ids = ids_flat[out_pos].astype(ids_flat.dtype)
    return out_scores, out_ids
